# revision 5
# baseline (speedup 1.0000x reference)
"""EngineOrderFFT (Bluestein chirp-Z, fixed M=16384) Trainium2 kernel.

Strategy
--------
Pure data parallelism: batch dim B=64 is split across 8 NeuronCores
(8 samples/core). Each sample's variable-length DFT (length n_b) is computed
as a Bluestein transform with fixed FFT size M=16384 = 128*128, and each
16384-point (i)FFT is a two-stage Cooley-Tukey factorization executed as
128x128 fp16 matmuls on the tensor engine:

    n = n1 + 128*n2, k = k2 + 128*k1
    X[k2+128*k1] = sum_n1 D[n1,k1] * W[n1,k2] * sum_n2 a[n1+128*n2] * D[n2,k2]

The inter-stage twiddle (o W) and the Bluestein pointwise products are
computed as 3 Karatsuba planes (P, Qn, R) on the vector engine; the complex
combine is absorbed into the next matmul's constant weights (PSUM
accumulation), so no separate add/sub passes are needed.

The final magnitude |conv[k]| equals |X[k]| (the output chirp has unit
modulus), so the kernel ships the complex conv planes (fp16, same bytes as
fp32 magnitudes) and the host takes hypot.

Host precompute (cheap, rpm-derived only): per-sample chirp tables
cos/-sin(pi*(t^2 mod 2n)/n), the FFT of the Bluestein kernel b (shipped as
Karatsuba planes, scaled 1/32), and the constant DFT/twiddle weight tables.
"""
import numpy as np

SF, RES, TS = 8192, 40, 1
B, L, C = 64, 8192, 8
M = 16384
NCORES = 8
SPC = B // NCORES  # samples per core

FBSCALE = 1.0 / 32.0
HSCALE = 1.0 / 16.0
KSCALE = 1.0 / 32.0  # HSCALE*KSCALE = (1/M) * (1/FBSCALE)

# ---------------------------------------------------------------------------
# constant tables (input-independent)
# ---------------------------------------------------------------------------


def _f16(x):
    return np.ascontiguousarray(x, dtype=np.float16)


def _build_const_tables():
    j = np.arange(128)
    D = np.exp(-2j * np.pi * np.outer(j, j) / 128.0)  # symmetric
    Dc = np.conj(D)
    Wt = np.exp(-2j * np.pi * np.outer(j, j) / M)  # fwd twiddle [n1,k2]
    W2 = np.conj(Wt)  # inv twiddle
    Dr, Di = D.real, D.imag
    Hr, Hi = (Dc * HSCALE).real, (Dc * HSCALE).imag
    Kr, Ki = (Dc * KSCALE).real[:, :64], (Dc * KSCALE).imag[:, :64]

    cols = []
    cols += [Dr + Di, Dr - Di, Di - Dr, -Di, Dr]  # G1,G2,G5,G3,G4 [128,128]
    cols += [np.concatenate([Hr + Hi, Hi - Hr], 1)]  # H_P [128,256]
    cols += [np.concatenate([Hr - Hi, Hi + Hr], 1)]  # H_Q
    cols += [np.concatenate([-Hi, Hr], 1)]  # H_R
    cols += [Wt.real, -Wt.imag, Wt.real + Wt.imag]  # Wr,Wni,Ws
    cols += [W2.real, -W2.imag, W2.real + W2.imag]  # W2r,W2ni,W2s
    cols += [Kr + Ki, Kr - Ki, Ki - Kr, -Ki, Kr]  # K1,K2,K5,K3,K4 [128,64]
    ca = _f16(np.concatenate(cols, axis=1))  # [128, 2496]

    cb = _f16(
        np.concatenate([Dr[:64], Di[:64], -Di[:64], Dr[:64]], axis=1)
    )  # [64, 512] = Dtab1|Dtab2
    return ca, cb


# column offsets in ca
_G = [0, 128, 256, 384, 512]  # G1,G2,G5,G3,G4
_HP, _HQ, _HR = 640, 896, 1152
_WR, _WNI, _WS = 1408, 1536, 1664
_W2R, _W2NI, _W2S = 1792, 1920, 2048
_K = [2176, 2240, 2304, 2368, 2432]  # K1,K2,K5,K3,K4
CA_COLS = 2496

_CONST_CACHE = {}


def _consts():
    if "ca" not in _CONST_CACHE:
        ca, cb = _build_const_tables()
        _CONST_CACHE["ca"] = ca
        _CONST_CACHE["cb"] = cb
    return _CONST_CACHE["ca"], _CONST_CACHE["cb"]


# ---------------------------------------------------------------------------
# device module
# ---------------------------------------------------------------------------

_MODULE_CACHE = {}


def _build_module():
    import concourse.bass as bass
    from concourse import mybir

    dt = mybir.dt
    NB = 2  # per-sample buffer depth

    nc = bass.Bass("TRN2", target_bir_lowering=False, debug=False)

    xt = nc.dram_tensor("xt", [SPC, C, L], dt.float32, kind="ExternalInput").ap()
    ach = nc.dram_tensor("ach", [SPC, 2, L], dt.float32, kind="ExternalInput").ap()
    fbd = nc.dram_tensor("fbd", [SPC, 3, 128, 128], dt.float16, kind="ExternalInput").ap()
    cad = nc.dram_tensor("cad", [128, CA_COLS], dt.float16, kind="ExternalInput").ap()
    cbd = nc.dram_tensor("cbd", [64, 512], dt.float16, kind="ExternalInput").ap()
    outr = nc.dram_tensor(
        "outr", [SPC, C, 2, L], dt.float16, kind="ExternalOutput"
    ).ap()

    ctx_list = []

    def sb(name, shape, dtype):
        t = nc.sbuf_tensor(name, shape, dtype)
        ap = t.__enter__()
        ctx_list.append(t)
        return ap

    def psum(name, shape, dtype):
        t = nc.psum_tensor(name, shape, dtype)
        ap = t.__enter__()
        ctx_list.append(t)
        return ap

    ca = sb("ca", [128, CA_COLS], dt.float16)
    cb = sb("cb", [64, 512], dt.float16)
    x_t = [sb(f"x{i}", [64, 1024], dt.float32) for i in range(NB)]
    # [64, 2048]: 8 channel pages of 128 (ch-major), t = n1 + 128*n2 grid
    ach_t = [sb(f"ach{i}", [64, 256], dt.float32) for i in range(NB)]
    fb_t = [sb(f"fb{i}", [128, 384], dt.float16) for i in range(NB)]
    A_t = [sb(f"A{i}", [64, 2048], dt.float16) for i in range(NB)]
    Yf = [sb(f"Yf{i}", [128, 2048], dt.float16) for i in range(NB)]
    PQ = [sb(f"PQ{i}", [128, 2048], dt.float16) for i in range(NB)]
    Rb = [sb(f"Rb{i}", [128, 1024], dt.float16) for i in range(NB)]
    S1 = [sb(f"S1{i}", [128, 1024], dt.float16) for i in range(NB)]
    Ff = [sb(f"Ff{i}", [128, 2048], dt.float16) for i in range(NB)]
    PQ2 = [sb(f"PQ2{i}", [128, 2048], dt.float16) for i in range(NB)]
    R2b = [sb(f"R2b{i}", [128, 1024], dt.float16) for i in range(NB)]
    S2 = [sb(f"S2{i}", [128, 1024], dt.float16) for i in range(NB)]
    Sf = [sb(f"Sf{i}", [128, 2048], dt.float16) for i in range(NB)]
    PQ3 = [sb(f"PQ3{i}", [128, 2048], dt.float16) for i in range(NB)]
    R3b = [sb(f"R3b{i}", [128, 1024], dt.float16) for i in range(NB)]
    S3 = [sb(f"S3{i}", [128, 1024], dt.float16) for i in range(NB)]
    ob = [sb(f"ob{i}", [64, 2048], dt.float16) for i in range(NB)]

    psA = psum("psA", [128, 2048], dt.float32)
    psB = psum("psB", [128, 2048], dt.float32)

    csem = nc.alloc_semaphore("csem")
    smp = [nc.alloc_semaphore(f"smp{i}") for i in range(SPC)]
    vsem = nc.alloc_semaphore("vsem")
    psem = nc.alloc_semaphore("psem")
    ssem = nc.alloc_semaphore("ssem")

    # static semaphore targets (14 DVE ops per sample, each +1 on vsem)
    def vA(s):
        return 14 * s + 2

    def vL1(s):
        return 14 * s + 6

    def vC(s):
        return 14 * s + 10

    def vL3(s):
        return 14 * s + 14

    def p1(s):
        return 4 * s + 1

    def p2(s):
        return 4 * s + 2

    def p3(s):
        return 4 * s + 3

    def p4(s):
        return 4 * s + 4

    def sY(s):
        return 4 * s + 1

    def sF(s):
        return 4 * s + 2

    def sS(s):
        return 4 * s + 3

    def sR(s):
        return 4 * s + 4

    AluOp = mybir.AluOpType
    AF = mybir.ActivationFunctionType

    with nc.Block() as block:

        @block.sync
        def _(sync):
            sync.dma_start(ca[:], cad[:]).then_inc(csem, 16)
            sync.dma_start(cb[:], cbd[:]).then_inc(csem, 16)

            def emit_in(s):
                b = s % NB
                if s >= NB:
                    sync.wait_ge(vsem, vC(s - NB))
                sync.dma_start(
                    x_t[b][:].rearrange("p (c n) -> p c n", c=C),
                    xt[s].rearrange("c (p n) -> p c n", n=128),
                ).then_inc(smp[s], 16)
                sync.dma_start(
                    ach_t[b][:].rearrange("p (r n) -> p r n", r=2),
                    ach[s].rearrange("r (p n) -> p r n", n=128),
                ).then_inc(smp[s], 16)
                sync.dma_start(
                    fb_t[b][:].rearrange("p (f n) -> p f n", f=3),
                    fbd[s].rearrange("f p n -> p f n"),
                ).then_inc(smp[s], 16)

            def emit_out(s):
                b = s % NB
                sync.wait_ge(ssem, sR(s))
                sync.dma_start(
                    outr[s].rearrange("c r (p n) -> p (c r) n", n=128),
                    ob[b][:].rearrange("p (q n) -> p q n", q=2 * C),
                ).then_inc(smp[s], 16)

            emit_in(0)
            emit_in(1)
            for s in range(SPC):
                if s + 2 < SPC:
                    emit_in(s + 2)
                emit_out(s)

        @block.vector
        def _(vector):
            def bcast64(ap128):
                return ap128.unsqueeze(1).broadcast_to([64, C, 128])

            def bcast128(ap128):
                return ap128.unsqueeze(1).broadcast_to([128, C, 128])

            def pages(ap, npage=C):
                return ap.rearrange("p (c n) -> p c n", c=npage)

            vector.wait_ge(csem, 32)
            for s in range(SPC):
                b = s % NB
                base = 14 * s
                vector.wait_ge(smp[s], 48)
                if s >= NB:
                    vector.wait_ge(psem, p1(s - NB))
                # a-planes: ar = x*cos, ai = x*(-sin)
                x3 = pages(x_t[b][:])
                nc.vector.tensor_tensor(
                    pages(A_t[b][:, 0:1024]), x3, bcast64(ach_t[b][:, 0:128]), AluOp.mult
                ).then_inc(vsem, 1)
                nc.vector.tensor_tensor(
                    pages(A_t[b][:, 1024:2048]),
                    x3,
                    bcast64(ach_t[b][:, 128:256]),
                    AluOp.mult,
                ).then_inc(vsem, 1)

                # L1 planes from evacuated Yf
                vector.wait_ge(ssem, sY(s))
                yre = pages(Yf[b][:])[:, :, 0:128]
                yim = pages(Yf[b][:])[:, :, 128:256]
                nc.vector.tensor_tensor(pages(S1[b][:]), yre, yim, AluOp.add).then_inc(
                    vsem, 1
                )
                nc.vector.tensor_tensor(
                    pages(PQ[b][:])[:, :, 0:128], yre, bcast128(ca[:, _WR : _WR + 128]), AluOp.mult
                ).then_inc(vsem, 1)
                nc.vector.tensor_tensor(
                    pages(PQ[b][:])[:, :, 128:256],
                    yim,
                    bcast128(ca[:, _WNI : _WNI + 128]),
                    AluOp.mult,
                ).then_inc(vsem, 1)
                vector.wait_ge(vsem, base + 3)  # s-plane drained (same-engine RAW)
                nc.vector.tensor_tensor(
                    pages(Rb[b][:]),
                    pages(S1[b][:]),
                    bcast128(ca[:, _WS : _WS + 128]),
                    AluOp.mult,
                ).then_inc(vsem, 1)

                # C-layer planes from evacuated Ff and Fb
                vector.wait_ge(ssem, sF(s))
                fre = pages(Ff[b][:])[:, :, 0:128]
                fim = pages(Ff[b][:])[:, :, 128:256]
                nc.vector.tensor_tensor(pages(S2[b][:]), fre, fim, AluOp.add).then_inc(
                    vsem, 1
                )
                nc.vector.tensor_tensor(
                    pages(PQ2[b][:])[:, :, 0:128], fre, bcast128(fb_t[b][:, 0:128]), AluOp.mult
                ).then_inc(vsem, 1)
                nc.vector.tensor_tensor(
                    pages(PQ2[b][:])[:, :, 128:256],
                    fim,
                    bcast128(fb_t[b][:, 128:256]),
                    AluOp.mult,
                ).then_inc(vsem, 1)
                vector.wait_ge(vsem, base + 7)
                nc.vector.tensor_tensor(
                    pages(R2b[b][:]),
                    pages(S2[b][:]),
                    bcast128(fb_t[b][:, 256:384]),
                    AluOp.mult,
                ).then_inc(vsem, 1)

                # L3 planes from evacuated Sf
                vector.wait_ge(ssem, sS(s))
                sre = pages(Sf[b][:])[:, :, 0:128]
                sim_ = pages(Sf[b][:])[:, :, 128:256]
                nc.vector.tensor_tensor(pages(S3[b][:]), sre, sim_, AluOp.add).then_inc(
                    vsem, 1
                )
                nc.vector.tensor_tensor(
                    pages(PQ3[b][:])[:, :, 0:128], sre, bcast128(ca[:, _W2R : _W2R + 128]), AluOp.mult
                ).then_inc(vsem, 1)
                nc.vector.tensor_tensor(
                    pages(PQ3[b][:])[:, :, 128:256],
                    sim_,
                    bcast128(ca[:, _W2NI : _W2NI + 128]),
                    AluOp.mult,
                ).then_inc(vsem, 1)
                vector.wait_ge(vsem, base + 11)
                nc.vector.tensor_tensor(
                    pages(R3b[b][:]),
                    pages(S3[b][:]),
                    bcast128(ca[:, _W2S : _W2S + 128]),
                    AluOp.mult,
                ).then_inc(vsem, 1)

        @block.tensor
        def _(tensor):
            mm = nc.tensor.matmul
            tensor.wait_ge(csem, 32)
            for s in range(SPC):
                b = s % NB
                # fwd stage 1: psA[ch] = [Ytr|Yti]
                tensor.wait_ge(vsem, vA(s))
                if s >= 1:
                    tensor.wait_ge(ssem, sS(s - 1))
                for c in range(C):
                    o = psA[:, 256 * c : 256 * c + 256]
                    last = c == C - 1
                    mm(o, A_t[b][:, 128 * c : 128 * c + 128], cb[:, 0:256], start=True, stop=False)
                    i = mm(
                        o,
                        A_t[b][:, 1024 + 128 * c : 1024 + 128 * c + 128],
                        cb[:, 256:512],
                        start=False,
                        stop=True,
                    )
                    if last:
                        i.then_inc(psem, 1)

                # fwd stage 2: psB[ch] = [Far|Fai]
                # cohorts of 4 (one open accumulation group per psum bank)
                tensor.wait_ge(vsem, vL1(s))
                if s >= 1:
                    tensor.wait_ge(ssem, sR(s - 1))
                for cohort in (range(0, C, 2), range(1, C, 2)):
                    for c in cohort:  # G1: [P|Qn] -> [re|im]
                        mm(
                            psB[:, 256 * c : 256 * c + 256],
                            ca[:, _G[0] : _G[0] + 128],
                            PQ[b][:, 256 * c : 256 * c + 256],
                            start=True,
                            stop=False,
                        )
                    for c in cohort:  # G2: Qn -> re
                        mm(
                            psB[:, 256 * c : 256 * c + 128],
                            ca[:, _G[1] : _G[1] + 128],
                            PQ[b][:, 256 * c + 128 : 256 * c + 256],
                            start=False,
                            stop=False,
                        )
                    for c in cohort:  # G5: P -> im
                        mm(
                            psB[:, 256 * c + 128 : 256 * c + 256],
                            ca[:, _G[2] : _G[2] + 128],
                            PQ[b][:, 256 * c : 256 * c + 128],
                            start=False,
                            stop=False,
                        )
                    for c in cohort:  # G3: R -> re
                        mm(
                            psB[:, 256 * c : 256 * c + 128],
                            ca[:, _G[3] : _G[3] + 128],
                            Rb[b][:, 128 * c : 128 * c + 128],
                            start=False,
                            stop=False,
                        )
                    for c in cohort:  # G4: R -> im (stop closes the bank group)
                        i = mm(
                            psB[:, 256 * c + 128 : 256 * c + 256],
                            ca[:, _G[4] : _G[4] + 128],
                            Rb[b][:, 128 * c : 128 * c + 128],
                            start=False,
                            stop=True,
                        )
                        if c == C - 1:
                            i.then_inc(psem, 1)

                # ifft stage 1: psA[ch] = [Sre|Sim], lhsT = C planes
                tensor.wait_ge(vsem, vC(s))
                for c in range(C):
                    o = psA[:, 256 * c : 256 * c + 256]
                    mm(
                        o,
                        PQ2[b][:, 256 * c : 256 * c + 128],
                        ca[:, _HP : _HP + 256],
                        start=True,
                        stop=False,
                    )
                    mm(
                        o,
                        PQ2[b][:, 256 * c + 128 : 256 * c + 256],
                        ca[:, _HQ : _HQ + 256],
                        start=False,
                        stop=False,
                    )
                    i = mm(
                        o,
                        R2b[b][:, 128 * c : 128 * c + 128],
                        ca[:, _HR : _HR + 256],
                        start=False,
                        stop=True,
                    )
                    if c == C - 1:
                        i.then_inc(psem, 1)

                # ifft stage 2: psB[0:64, ch] = [Rr|Ri]
                tensor.wait_ge(vsem, vL3(s))
                for cohort in (range(0, C, 2), range(1, C, 2)):
                    for c in cohort:  # K1: [P3|Q3n] -> [re|im]
                        mm(
                            psB[0:64, 256 * c : 256 * c + 256],
                            ca[:, _K[0] : _K[0] + 64],
                            PQ3[b][:, 256 * c : 256 * c + 256],
                            start=True,
                            stop=False,
                        )
                    for c in cohort:  # K2: Q3n -> re
                        mm(
                            psB[0:64, 256 * c : 256 * c + 128],
                            ca[:, _K[1] : _K[1] + 64],
                            PQ3[b][:, 256 * c + 128 : 256 * c + 256],
                            start=False,
                            stop=False,
                        )
                    for c in cohort:  # K5: P3 -> im
                        mm(
                            psB[0:64, 256 * c + 128 : 256 * c + 256],
                            ca[:, _K[2] : _K[2] + 64],
                            PQ3[b][:, 256 * c : 256 * c + 128],
                            start=False,
                            stop=False,
                        )
                    for c in cohort:  # K3: R3 -> re
                        mm(
                            psB[0:64, 256 * c : 256 * c + 128],
                            ca[:, _K[3] : _K[3] + 64],
                            R3b[b][:, 128 * c : 128 * c + 128],
                            start=False,
                            stop=False,
                        )
                    for c in cohort:  # K4: R3 -> im (stop closes the bank group)
                        i = mm(
                            psB[0:64, 256 * c + 128 : 256 * c + 256],
                            ca[:, _K[4] : _K[4] + 64],
                            R3b[b][:, 128 * c : 128 * c + 128],
                            start=False,
                            stop=True,
                        )
                        if c == C - 1:
                            i.then_inc(psem, 1)

        @block.scalar
        def _(scalar):
            for s in range(SPC):
                b = s % NB
                scalar.wait_ge(psem, p1(s))
                nc.scalar.copy(Yf[b][:], psA[:, 0:2048]).then_inc(ssem, 1)
                scalar.wait_ge(psem, p2(s))
                nc.scalar.copy(Ff[b][:], psB[:, 0:2048]).then_inc(ssem, 1)
                scalar.wait_ge(psem, p3(s))
                nc.scalar.copy(Sf[b][:], psA[:, 0:2048]).then_inc(ssem, 1)
                scalar.wait_ge(psem, p4(s))
                if s >= NB:
                    scalar.wait_ge(smp[s - NB], 64)
                nc.scalar.copy(ob[b][:], psB[0:64, 0:2048]).then_inc(ssem, 1)

    for t in reversed(ctx_list):
        t.__exit__(None, None, None)

    return nc


def _get_module():
    if "nc" not in _MODULE_CACHE:
        _MODULE_CACHE["nc"] = _build_module()
    return _MODULE_CACHE["nc"]


# ---------------------------------------------------------------------------
# host side
# ---------------------------------------------------------------------------


def _host_tables(rpm):
    """Per-sample chirp tables + Fb planes. rpm: [B] float32."""
    pad = np.floor((RES * 60.0 / rpm.astype(np.float64) - TS) * SF).astype(np.int64)
    n_arr = L + pad
    t = np.arange(L, dtype=np.int64)
    m = np.arange(M, dtype=np.int64)
    mm = np.minimum(m, M - m)

    ach = np.empty((B, 2, L), np.float32)
    fbp = np.empty((B, 3, 128, 128), np.float16)
    for b in range(B):
        n = int(n_arr[b])
        two_n = 2 * n
        ph = np.pi * ((t * t) % two_n) / n
        ach[b, 0] = np.cos(ph)
        ach[b, 1] = -np.sin(ph)
        phb = np.pi * ((mm * mm) % two_n) / n
        Fb = np.fft.fft(np.exp(1j * phb)).reshape(128, 128) * FBSCALE
        fbp[b, 0] = Fb.real.astype(np.float16)
        fbp[b, 1] = (-Fb.imag).astype(np.float16)
        fbp[b, 2] = (Fb.real + Fb.imag).astype(np.float16)
    return ach, fbp


LAST_EXEC_WALL_NS = [None]


def kernel(inputs, rpm):
    inputs = np.ascontiguousarray(inputs, dtype=np.float32)  # [B, L, C]
    rpm = np.ascontiguousarray(rpm, dtype=np.float32)

    ca, cb = _consts()
    ach, fbp = _host_tables(rpm)
    xt = np.ascontiguousarray(inputs.transpose(0, 2, 1))  # [B, C, L]

    nc = _get_module()
    in_maps = []
    for g in range(NCORES):
        s0 = g * SPC
        in_maps.append(
            {
                "xt": xt[s0 : s0 + SPC],
                "ach": ach[s0 : s0 + SPC],
                "fbd": fbp[s0 : s0 + SPC],
                "cad": ca,
                "cbd": cb,
            }
        )

    import time

    from concourse.bass_utils import run_bass_kernel_spmd

    t0 = time.perf_counter_ns()
    res = run_bass_kernel_spmd(nc, in_maps, list(range(NCORES)))
    LAST_EXEC_WALL_NS[0] = time.perf_counter_ns() - t0

    out = np.empty((B, L, C), np.float32)
    for g in range(NCORES):
        planes = np.asarray(res.results[g]["outr"], np.float32)  # [SPC, C, 2, L]
        mag = np.hypot(planes[:, :, 0, :], planes[:, :, 1, :])  # [SPC, C, L]
        out[g * SPC : (g + 1) * SPC] = mag.transpose(0, 2, 1)
    return out


# revision 17
# speedup vs baseline: 8135.5545x; 8135.5545x over previous
"""EngineOrderFFT (Bluestein chirp-Z, fixed M=16384) Trainium2 kernel.

Strategy
--------
Pure data parallelism: batch dim B=64 is split across 8 NeuronCores
(8 samples/core). Each sample's variable-length DFT (length n_b) is computed
as a Bluestein transform with fixed FFT size M=16384 = 128*128, and each
16384-point (i)FFT is a two-stage Cooley-Tukey factorization executed as
128x128 fp16 matmuls on the tensor engine:

    n = n1 + 128*n2, k = k2 + 128*k1
    X[k2+128*k1] = sum_n1 D[n1,k1] * W[n1,k2] * sum_n2 a[n1+128*n2] * D[n2,k2]

Engine split per sample (8 channels batched in every instruction):
  sync   in/out DMAs
  gpsimd a-planes (x*chirp) + Fb plane replication across channel pages
  PE     4 matmul stages; twiddle/pointwise complex combines are absorbed
         into constant weights via PSUM accumulation (Karatsuba planes for
         the two twiddle layers, plain products for the Fa*Fb layer)
  ACT    PSUM -> fp16 SBUF evacuation after each stage
  DVE    twiddle/pointwise product planes (fp16 2x mode, replicated tables)

The final magnitude |conv[k]| equals |X[k]| (the output chirp has unit
modulus), so the kernel ships the complex conv planes (fp16, same bytes as
fp32 magnitudes) and the host takes hypot.

Host precompute (cheap, rpm-derived only): per-sample chirp tables
cos/-sin(pi*(t^2 mod 2n)/n), the FFT of the Bluestein kernel b (scaled
1/32), and the constant DFT/twiddle weight tables.
"""
import numpy as np

SF, RES, TS = 8192, 40, 1
B, L, C = 64, 8192, 8
M = 16384
NCORES = 8
SPC = B // NCORES  # samples per core

FBSCALE = 1.0 / 32.0
HSCALE = 1.0 / 16.0
KSCALE = 1.0 / 32.0  # HSCALE*KSCALE = (1/M) * (1/FBSCALE)

# ---------------------------------------------------------------------------
# constant tables (input-independent)
# ---------------------------------------------------------------------------


def _f16(x):
    return np.ascontiguousarray(x, dtype=np.float16)


def _rep8(x):
    return np.tile(x, (1, C))


def _build_const_tables():
    j = np.arange(128)
    D = np.exp(-2j * np.pi * np.outer(j, j) / 128.0)  # symmetric
    Dc = np.conj(D)
    Wt = np.exp(-2j * np.pi * np.outer(j, j) / M)  # fwd twiddle [n1,k2]
    W2 = np.conj(Wt)  # inv twiddle
    Dr, Di = D.real, D.imag
    Hr, Hi = (Dc * HSCALE).real, (Dc * HSCALE).imag
    Kr, Ki = (Dc * KSCALE).real[:, :64], (Dc * KSCALE).imag[:, :64]

    cols = []
    # F (fwd stage2, Karatsuba combine): F1,F2,F2n,F3,F4 [128,128]
    cols += [Dr + Di, Dr - Di, Di - Dr, -Di, Dr]
    # H (ifft stage1, plain complex): H_A=[Hr|Hi], H_B=[-Hi|Hr] [128,256]
    cols += [np.concatenate([Hr, Hi], 1)]
    cols += [np.concatenate([-Hi, Hr], 1)]
    # K (ifft stage2, Karatsuba): K1,K2,K2n,K3,K4 [128,64]
    cols += [Kr + Ki, Kr - Ki, Ki - Kr, -Ki, Kr]
    # twiddle tables replicated x8 channel pages (keeps DVE in 2x mode)
    cols += [_rep8(Wt.real), _rep8(-Wt.imag), _rep8(Wt.real + Wt.imag)]
    cols += [_rep8(W2.real), _rep8(-W2.imag), _rep8(W2.real + W2.imag)]
    ca = _f16(np.concatenate(cols, axis=1))

    cb = _f16(
        np.concatenate([Dr[:64], Di[:64], -Di[:64], Dr[:64]], axis=1)
    )  # [64, 512] = Dtab1|Dtab2
    return ca, cb


# column offsets in ca
_F = [0, 128, 256, 384, 512]  # F1,F2,F2n,F3,F4
_HA, _HB = 640, 896
_K = [1152, 1216, 1280, 1344, 1408]  # K1,K2,K2n,K3,K4
_WR, _WNI, _WS = 1472, 2496, 3520  # [128, 1024] each (replicated x8)
_W2R, _W2NI, _W2S = 4544, 5568, 6592
CA1_COLS = 4544
CA_COLS = 7616

_CONST_CACHE = {}


def _consts():
    if "ca" not in _CONST_CACHE:
        ca, cb = _build_const_tables()
        assert ca.shape[1] == CA_COLS, ca.shape
        _CONST_CACHE["ca"] = ca
        _CONST_CACHE["cb"] = cb
    return _CONST_CACHE["ca"], _CONST_CACHE["cb"]


# ---------------------------------------------------------------------------
# device module
# ---------------------------------------------------------------------------

_MODULE_CACHE = {}


def _build_module():
    import concourse.bass as bass
    from concourse import mybir

    dt = mybir.dt
    NB = 2  # per-sample buffer depth

    nc = bass.Bass("TRN2", target_bir_lowering=False, debug=False)

    xt = nc.dram_tensor("xt", [SPC, C, L], dt.float16, kind="ExternalInput").ap()
    # chirp tables, already replicated x8 channels: [SPC, 2, 64, 1024]
    ach = nc.dram_tensor("ach", [SPC, 2, 64, 1024], dt.float16, kind="ExternalInput").ap()
    # Fb planes (replicated x8 ch): [SPC, 3, 128, 1024] = (Fbr, +Fbi, -Fbi)*FBSCALE
    fbd = nc.dram_tensor("fbd", [SPC, 3, 128, 1024], dt.float16, kind="ExternalInput").ap()
    cad = nc.dram_tensor("cad", [128, CA_COLS], dt.float16, kind="ExternalInput").ap()
    cbd = nc.dram_tensor("cbd", [64, 512], dt.float16, kind="ExternalInput").ap()
    outr = nc.dram_tensor(
        "outr", [SPC, C, 2, L], dt.float16, kind="ExternalOutput"
    ).ap()

    ctx_list = []

    def sb(name, shape, dtype=None):
        t = nc.sbuf_tensor(name, shape, dtype or mybir.dt.float16)
        ap = t.__enter__()
        ctx_list.append(t)
        return ap

    def psum(name, shape):
        t = nc.psum_tensor(name, shape, mybir.dt.float32)
        ap = t.__enter__()
        ctx_list.append(t)
        return ap

    ca = sb("ca", [128, CA_COLS])
    cb = sb("cb", [64, 512])
    x_t = [sb(f"x{i}", [64, 1024]) for i in range(NB)]
    ach_t = [sb(f"ach{i}", [64, 2048]) for i in range(NB)]
    fbR = [sb(f"fbR{i}", [128, 3072]) for i in range(NB)]
    A_t = [sb(f"A{i}", [64, 2048]) for i in range(NB)]
    Yf = [sb(f"Yf{i}", [128, 2048]) for i in range(NB)]
    Pb = [sb(f"Pb{i}", [128, 1024]) for i in range(NB)]
    Qnb = [sb(f"Qnb{i}", [128, 1024]) for i in range(NB)]
    Rb = [sb(f"Rb{i}", [128, 1024]) for i in range(NB)]
    Ff = [sb(f"Ff{i}", [128, 2048]) for i in range(NB)]
    CRb = [sb(f"CRb{i}", [128, 1024]) for i in range(NB)]
    CIb = [sb(f"CIb{i}", [128, 1024]) for i in range(NB)]
    Sf = [sb(f"Sf{i}", [128, 2048]) for i in range(NB)]
    P3b = [sb(f"P3b{i}", [128, 1024]) for i in range(NB)]
    Q3nb = [sb(f"Q3nb{i}", [128, 1024]) for i in range(NB)]
    R3b = [sb(f"R3b{i}", [128, 1024]) for i in range(NB)]
    ob = [sb(f"ob{i}", [64, 2048]) for i in range(NB)]
    # single-buffered DVE scratch (same-engine producer/consumer) -- but with
    # pair interleaving two samples' DVE groups are adjacent, so double them
    S1 = [sb(f"S1_{i}", [128, 1024]) for i in range(NB)]
    S3 = [sb(f"S3_{i}", [128, 1024]) for i in range(NB)]
    M1 = [sb(f"M1_{i}", [128, 1024]) for i in range(NB)]
    M2 = [sb(f"M2_{i}", [128, 1024]) for i in range(NB)]
    M3 = [sb(f"M3_{i}", [128, 1024]) for i in range(NB)]
    M4 = [sb(f"M4_{i}", [128, 1024]) for i in range(NB)]

    # two 4-bank psum regions; samples alternate regions by parity, and each
    # region runs its own strict phase sequence s1 -> s2 -> is1 -> is2
    psR = [psum("psR0", [128, 2048]), psum("psR1", [128, 2048])]

    csem = nc.alloc_semaphore("csem")
    c2sem = nc.alloc_semaphore("c2sem")
    cbsem = nc.alloc_semaphore("cbsem")
    smp = [nc.alloc_semaphore(f"smp{i}") for i in range(SPC)]
    osem = [nc.alloc_semaphore(f"osem{i}") for i in range(SPC)]
    fsem = [nc.alloc_semaphore(f"fsem{i}") for i in range(SPC)]
    vsem = nc.alloc_semaphore("vsem")
    psem = nc.alloc_semaphore("psem")
    ssem = nc.alloc_semaphore("ssem")
    gsem = nc.alloc_semaphore("gsem")

    # ---- emission orders (pair-interleaved) and semaphore target tables ----
    pairs = [(2 * p, 2 * p + 1) for p in range(SPC // 2)]

    pe_order = []   # (phase, s), phase in 0..3
    act_order = []  # (evac, s)
    dve_order = []  # (group, s), group in 0..2 (L1, CL, L3)
    gp_order = []   # (kind, s), kind 0=a-planes, 1=fbR
    for (sa, sb_) in pairs:
        for ph in range(4):
            pe_order += [(ph, sa), (ph, sb_)]
            act_order += [(ph, sa), (ph, sb_)]
        for g in range(3):
            dve_order += [(g, sa), (g, sb_)]
        gp_order += [(0, sa), (0, sb_)]

    PSEM = {}
    for i, key in enumerate(pe_order):
        PSEM[key] = i + 1
    SSEM = {}
    for i, key in enumerate(act_order):
        SSEM[key] = i + 1
    GSEM = {}
    g = 0
    for kind, s in gp_order:
        g += 2
        GSEM[(kind, s)] = g
    # DVE op positions per group (emission order below):
    #  L1/L3: P(+1), Qn(+2), S(+3), R(+4);  CL: M1,M2,CR,M3,M4,CI (+1..+6)
    VSEM = {}
    VOP = {}
    v = 0
    for grp, s in dve_order:
        nops = (4, 6, 4)[grp]
        for k in range(1, nops + 1):
            VOP[(grp, s, k)] = v + k
        v += nops
        VSEM[(grp, s)] = v

    AluOp = mybir.AluOpType

    with nc.Block() as block:

        @block.sync
        def _(sync):
            def emit_in(s):
                b = s % NB
                if s >= NB:
                    sync.wait_ge(gsem, GSEM[(0, s - NB)])
                    sync.wait_ge(vsem, VSEM[(1, s - NB)])
                sync.dma_start(
                    x_t[b][:].rearrange("p (c n) -> p c n", c=C),
                    xt[s].rearrange("c (p n) -> p c n", n=128),
                ).then_inc(smp[s], 16)
                sync.dma_start(
                    ach_t[b][:].rearrange("p (r n) -> p r n", r=2),
                    ach[s].rearrange("r p n -> p r n"),
                ).then_inc(smp[s], 16)
                sync.dma_start(
                    fbR[b][:].rearrange("p (f n) -> p f n", f=3),
                    fbd[s].rearrange("f p n -> p f n"),
                ).then_inc(fsem[s], 16)

            def emit_out(s):
                b = s % NB
                sync.wait_ge(ssem, SSEM[(3, s)])
                obv = ob[b][:].rearrange("p (q v) -> p q v", q=4)
                orv = outr[s].rearrange("(q j) r (p n) -> p q j r n", q=4, n=128)
                for r in range(2):
                    for jj in range(2):
                        sync.dma_start(
                            orv[:, :, jj, r, :],
                            obv[:, :, 256 * r + 128 * jj : 256 * r + 128 * jj + 128],
                        ).then_inc(osem[s], 16)

            # startup: tiny cb first, then sample-0 x/ach so Pool can start,
            # then the two halves of the big constant table between loads
            sync.dma_start(cb[:], cbd[:]).then_inc(cbsem, 16)
            sync.dma_start(
                x_t[0][:].rearrange("p (c n) -> p c n", c=C),
                xt[0].rearrange("c (p n) -> p c n", n=128),
            ).then_inc(smp[0], 16)
            sync.dma_start(
                ach_t[0][:].rearrange("p (r n) -> p r n", r=2),
                ach[0].rearrange("r p n -> p r n"),
            ).then_inc(smp[0], 16)
            sync.dma_start(ca[:, 0:CA1_COLS], cad[:, 0:CA1_COLS]).then_inc(csem, 16)
            sync.dma_start(
                fbR[0][:].rearrange("p (f n) -> p f n", f=3),
                fbd[0].rearrange("f p n -> p f n"),
            ).then_inc(fsem[0], 16)
            emit_in(1)
            sync.dma_start(ca[:, CA1_COLS:], cad[:, CA1_COLS:]).then_inc(c2sem, 16)
            emit_in(2)
            emit_in(3)
            for s in range(SPC):
                if s + 4 < SPC:
                    emit_in(s + 4)
                emit_out(s)

        @block.gpsimd
        def _(gp):
            for kind, s in gp_order:
                b = s % NB
                gp.wait_ge(smp[s], 32)
                if s >= NB:
                    gp.wait_ge(psem, PSEM[(0, s - NB)])  # A_t[b] free
                nc.gpsimd.tensor_tensor(
                    A_t[b][:, 0:1024], x_t[b][:], ach_t[b][:, 0:1024], AluOp.mult
                ).then_inc(gsem, 1)
                nc.gpsimd.tensor_tensor(
                    A_t[b][:, 1024:2048],
                    x_t[b][:],
                    ach_t[b][:, 1024:2048],
                    AluOp.mult,
                ).then_inc(gsem, 1)

        @block.vector
        def _(vector):
            def chpages(ap):
                v_ = ap.rearrange("p (c u) -> p c u", c=C)
                return v_[:, :, 0:128], v_[:, :, 128:256]

            def prpages(ap):
                v_ = ap.rearrange("p (q u) -> p q u", q=4)
                return v_[:, :, 0:256], v_[:, :, 256:512]

            def flat4(ap):
                return ap.rearrange("p (q u) -> p q u", q=4)

            def flat8(ap):
                return ap.rearrange("p (c u) -> p c u", c=C)

            first_dve = [True]
            for grp, s in dve_order:
                if first_dve[0]:
                    vector.wait_ge(csem, 16)
                    first_dve[0] = False
                    first_l3 = [True]
                b = s % NB
                if grp == 0:
                    # L1 (fwd twiddle, Karatsuba planes) from Yf
                    vector.wait_ge(ssem, SSEM[(0, s)])
                    if s >= NB:
                        vector.wait_ge(psem, PSEM[(1, s - NB)])  # Pb/Qnb/Rb free
                    yre, yim = chpages(Yf[b][:])
                    nc.vector.tensor_tensor(
                        flat8(Pb[b][:]), yre, flat8(ca[:, _WR : _WR + 1024]), AluOp.mult
                    ).then_inc(vsem, 1)
                    nc.vector.tensor_tensor(
                        flat8(Qnb[b][:]),
                        yim,
                        flat8(ca[:, _WNI : _WNI + 1024]),
                        AluOp.mult,
                    ).then_inc(vsem, 1)
                    nc.vector.tensor_tensor(
                        flat8(S1[b][:]), yre, yim, AluOp.add
                    ).then_inc(vsem, 1)
                    vector.wait_ge(vsem, VOP[(grp, s, 3)])  # S1 drained
                    nc.vector.tensor_tensor(
                        Rb[b][:], S1[b][:], ca[:, _WS : _WS + 1024], AluOp.mult
                    ).then_inc(vsem, 1)
                elif grp == 1:
                    # C-layer (Fa o Fb, plain complex) from Ff (pair-major)
                    vector.wait_ge(ssem, SSEM[(1, s)])
                    vector.wait_ge(fsem[s], 16)
                    fre, fim = prpages(Ff[b][:])
                    nc.vector.tensor_tensor(
                        flat4(M1[b][:]), fre, flat4(fbR[b][:, 0:1024]), AluOp.mult
                    ).then_inc(vsem, 1)
                    nc.vector.tensor_tensor(
                        flat4(M2[b][:]), fim, flat4(fbR[b][:, 2048:3072]), AluOp.mult
                    ).then_inc(vsem, 1)
                    vector.wait_ge(vsem, VOP[(grp, s, 2)])  # M1, M2 drained
                    if s >= NB:
                        vector.wait_ge(psem, PSEM[(2, s - NB)])  # CRb/CIb free
                    nc.vector.tensor_tensor(
                        CRb[b][:], M1[b][:], M2[b][:], AluOp.add
                    ).then_inc(vsem, 1)
                    nc.vector.tensor_tensor(
                        flat4(M3[b][:]), fre, flat4(fbR[b][:, 1024:2048]), AluOp.mult
                    ).then_inc(vsem, 1)
                    nc.vector.tensor_tensor(
                        flat4(M4[b][:]), fim, flat4(fbR[b][:, 0:1024]), AluOp.mult
                    ).then_inc(vsem, 1)
                    vector.wait_ge(vsem, VOP[(grp, s, 5)])  # M3, M4 drained
                    nc.vector.tensor_tensor(
                        CIb[b][:], M3[b][:], M4[b][:], AluOp.add
                    ).then_inc(vsem, 1)
                else:
                    # L3 (inv twiddle, Karatsuba planes) from Sf
                    if first_l3[0]:
                        vector.wait_ge(c2sem, 16)
                        first_l3[0] = False
                    vector.wait_ge(ssem, SSEM[(2, s)])
                    if s >= NB:
                        vector.wait_ge(psem, PSEM[(3, s - NB)])  # P3b/.. free
                    sre, sim_ = chpages(Sf[b][:])
                    nc.vector.tensor_tensor(
                        flat8(P3b[b][:]),
                        sre,
                        flat8(ca[:, _W2R : _W2R + 1024]),
                        AluOp.mult,
                    ).then_inc(vsem, 1)
                    nc.vector.tensor_tensor(
                        flat8(Q3nb[b][:]),
                        sim_,
                        flat8(ca[:, _W2NI : _W2NI + 1024]),
                        AluOp.mult,
                    ).then_inc(vsem, 1)
                    nc.vector.tensor_tensor(
                        flat8(S3[b][:]), sre, sim_, AluOp.add
                    ).then_inc(vsem, 1)
                    vector.wait_ge(vsem, VOP[(grp, s, 3)])  # S3 drained
                    nc.vector.tensor_tensor(
                        R3b[b][:], S3[b][:], ca[:, _W2S : _W2S + 1024], AluOp.mult
                    ).then_inc(vsem, 1)

        @block.tensor
        def _(tensor):
            mm = nc.tensor.matmul
            first_pe = [True]

            def phase_s1(s):
                b = s % NB
                ps = psR[s % 2]
                if first_pe[0]:
                    tensor.wait_ge(cbsem, 16)  # cb loaded
                    first_pe[0] = False
                tensor.wait_ge(gsem, GSEM[(0, s)])
                if s >= NB:
                    tensor.wait_ge(ssem, SSEM[(3, s - NB)])  # region free
                for c in range(C):
                    o = ps[:, 256 * c : 256 * c + 256]
                    mm(
                        o,
                        A_t[b][:, 128 * c : 128 * c + 128],
                        cb[:, 0:256],
                        start=True,
                        stop=False,
                    )
                    i = mm(
                        o,
                        A_t[b][:, 1024 + 128 * c : 1024 + 128 * c + 128],
                        cb[:, 256:512],
                        start=False,
                        stop=True,
                    )
                    if c == C - 1:
                        i.then_inc(psem, 1)

            def phase_s2(s):
                b = s % NB
                ps = psR[s % 2]
                tensor.wait_ge(vsem, VOP[(0, s, 1)])  # Pb ready
                tensor.wait_ge(csem, 16)  # ca loaded
                tensor.wait_ge(ssem, SSEM[(0, s)])  # region free after evacY
                srcs = [
                    (Pb[b], _F[0], 0, True, False, None),
                    (Qnb[b], _F[0], 256, False, False, VOP[(0, s, 2)]),
                    (Qnb[b], _F[1], 0, False, False, None),
                    (Pb[b], _F[2], 256, False, False, None),
                    (Rb[b], _F[3], 0, False, False, VOP[(0, s, 4)]),
                    (Rb[b], _F[4], 256, False, True, None),
                ]
                for wi, (buf, fofs, oofs, st, sp, wv) in enumerate(srcs):
                    if wv is not None:
                        tensor.wait_ge(vsem, wv)
                    for q in range(4):
                        i = mm(
                            ps[:, 512 * q + oofs : 512 * q + oofs + 256],
                            ca[:, fofs : fofs + 128],
                            buf[:, 256 * q : 256 * q + 256],
                            start=st,
                            stop=sp,
                        )
                        if wi == 5 and q == 3:
                            i.then_inc(psem, 1)

            def phase_is1(s):
                b = s % NB
                ps = psR[s % 2]
                tensor.wait_ge(vsem, VOP[(1, s, 3)])  # CRb ready
                tensor.wait_ge(ssem, SSEM[(1, s)])
                for c in range(C):
                    # even channel opens its bank; odd writes the other half
                    mm(
                        ps[:, 256 * c : 256 * c + 256],
                        CRb[b][:, 128 * c : 128 * c + 128],
                        ca[:, _HA : _HA + 256],
                        start=(c % 2 == 0),
                        stop=False,
                    )
                tensor.wait_ge(vsem, VOP[(1, s, 6)])  # CIb ready
                for c in range(C):
                    i = mm(
                        ps[:, 256 * c : 256 * c + 256],
                        CIb[b][:, 128 * c : 128 * c + 128],
                        ca[:, _HB : _HB + 256],
                        start=False,
                        stop=(c % 2 == 1),
                    )
                    if c == C - 1:
                        i.then_inc(psem, 1)

            def phase_is2(s):
                b = s % NB
                ps = psR[s % 2]
                tensor.wait_ge(vsem, VOP[(2, s, 1)])  # P3b ready
                tensor.wait_ge(ssem, SSEM[(2, s)])
                srcs = [
                    (P3b[b], _K[0], 0, True, False, None),
                    (Q3nb[b], _K[0], 256, False, False, VOP[(2, s, 2)]),
                    (Q3nb[b], _K[1], 0, False, False, None),
                    (P3b[b], _K[2], 256, False, False, None),
                    (R3b[b], _K[3], 0, False, False, VOP[(2, s, 4)]),
                    (R3b[b], _K[4], 256, False, True, None),
                ]
                for wi, (buf, kofs, oofs, st, sp, wv) in enumerate(srcs):
                    if wv is not None:
                        tensor.wait_ge(vsem, wv)
                    for q in range(4):
                        i = mm(
                            ps[0:64, 512 * q + oofs : 512 * q + oofs + 256],
                            ca[:, kofs : kofs + 64],
                            buf[:, 256 * q : 256 * q + 256],
                            start=st,
                            stop=sp,
                        )
                        if wi == 5 and q == 3:
                            i.then_inc(psem, 1)

            phase_fns = [phase_s1, phase_s2, phase_is1, phase_is2]
            for ph, s in pe_order:
                phase_fns[ph](s)

        @block.scalar
        def _(scalar):
            for ph, s in act_order:
                b = s % NB
                ps = psR[s % 2]
                scalar.wait_ge(psem, PSEM[(ph, s)])
                if ph == 0:
                    nc.scalar.copy(Yf[b][:], ps[:, 0:2048]).then_inc(ssem, 1)
                elif ph == 1:
                    nc.scalar.copy(Ff[b][:], ps[:, 0:2048]).then_inc(ssem, 1)
                elif ph == 2:
                    nc.scalar.copy(Sf[b][:], ps[:, 0:2048]).then_inc(ssem, 1)
                else:
                    if s >= NB:
                        scalar.wait_ge(osem[s - NB], 64)
                    nc.scalar.copy(ob[b][:], ps[0:64, 0:2048]).then_inc(ssem, 1)

    for t in reversed(ctx_list):
        t.__exit__(None, None, None)

    return nc


def _get_module():
    if "nc" not in _MODULE_CACHE:
        _MODULE_CACHE["nc"] = _build_module()
    return _MODULE_CACHE["nc"]


# ---------------------------------------------------------------------------
# host side
# ---------------------------------------------------------------------------


def _host_tables(rpm):
    """Per-sample chirp tables (replicated x8 channels) + Fb planes."""
    pad = np.floor((RES * 60.0 / rpm.astype(np.float64) - TS) * SF).astype(np.int64)
    n_arr = L + pad
    t = np.arange(L, dtype=np.int64)
    m = np.arange(M, dtype=np.int64)
    mm = np.minimum(m, M - m)

    ach = np.empty((B, 2, 64, 1024), np.float16)
    fbp = np.empty((B, 3, 128, 1024), np.float16)
    for b in range(B):
        n = int(n_arr[b])
        two_n = 2 * n
        ph = np.pi * ((t * t) % two_n) / n
        cosg = np.cos(ph).astype(np.float16).reshape(64, 128)
        nsing = (-np.sin(ph)).astype(np.float16).reshape(64, 128)
        ach[b, 0] = np.tile(cosg, (1, C))
        ach[b, 1] = np.tile(nsing, (1, C))
        phb = np.pi * ((mm * mm) % two_n) / n
        Fb = np.fft.fft(np.exp(1j * phb)).reshape(128, 128) * FBSCALE
        fbp[b, 0] = np.tile(Fb.real.astype(np.float16), (1, C))
        fbp[b, 1] = np.tile(Fb.imag.astype(np.float16), (1, C))
        fbp[b, 2] = np.tile((-Fb.imag).astype(np.float16), (1, C))
    return ach, fbp


LAST_EXEC_WALL_NS = [None]


def kernel(inputs, rpm):
    inputs = np.ascontiguousarray(inputs, dtype=np.float32)  # [B, L, C]
    rpm = np.ascontiguousarray(rpm, dtype=np.float32)

    ca, cb = _consts()
    ach, fbp = _host_tables(rpm)
    xt = np.ascontiguousarray(inputs.transpose(0, 2, 1)).astype(np.float16)

    nc = _get_module()
    in_maps = []
    for g in range(NCORES):
        s0 = g * SPC
        in_maps.append(
            {
                "xt": xt[s0 : s0 + SPC],
                "ach": ach[s0 : s0 + SPC],
                "fbd": fbp[s0 : s0 + SPC],
                "cad": ca,
                "cbd": cb,
            }
        )

    import time

    from concourse.bass_utils import run_bass_kernel_spmd

    t0 = time.perf_counter_ns()
    res = run_bass_kernel_spmd(nc, in_maps, list(range(NCORES)))
    LAST_EXEC_WALL_NS[0] = time.perf_counter_ns() - t0

    out = np.empty((B, L, C), np.float32)
    for g in range(NCORES):
        planes = np.asarray(res.results[g]["outr"], np.float32)  # [SPC, C, 2, L]
        mag = np.hypot(planes[:, :, 0, :], planes[:, :, 1, :])  # [SPC, C, L]
        out[g * SPC : (g + 1) * SPC] = mag.transpose(0, 2, 1)
    return out


# revision 18
# speedup vs baseline: 8334.9283x; 1.0245x over previous
"""EngineOrderFFT (Bluestein chirp-Z, fixed M=16384) Trainium2 kernel.

Strategy
--------
Pure data parallelism: batch dim B=64 is split across 8 NeuronCores
(8 samples/core). Each sample's variable-length DFT (length n_b) is computed
as a Bluestein transform with fixed FFT size M=16384 = 128*128, and each
16384-point (i)FFT is a two-stage Cooley-Tukey factorization executed as
128x128 fp16 matmuls on the tensor engine:

    n = n1 + 128*n2, k = k2 + 128*k1
    X[k2+128*k1] = sum_n1 D[n1,k1] * W[n1,k2] * sum_n2 a[n1+128*n2] * D[n2,k2]

Engine split per sample (8 channels batched in every instruction):
  sync   in/out DMAs
  gpsimd a-planes (x*chirp) + Fb plane replication across channel pages
  PE     4 matmul stages; twiddle/pointwise complex combines are absorbed
         into constant weights via PSUM accumulation (Karatsuba planes for
         the two twiddle layers, plain products for the Fa*Fb layer)
  ACT    PSUM -> fp16 SBUF evacuation after each stage
  DVE    twiddle/pointwise product planes (fp16 2x mode, replicated tables)

The final magnitude |conv[k]| equals |X[k]| (the output chirp has unit
modulus), so the kernel ships the complex conv planes (fp16, same bytes as
fp32 magnitudes) and the host takes hypot.

Host precompute (cheap, rpm-derived only): per-sample chirp tables
cos/-sin(pi*(t^2 mod 2n)/n), the FFT of the Bluestein kernel b (scaled
1/32), and the constant DFT/twiddle weight tables.
"""
import numpy as np

SF, RES, TS = 8192, 40, 1
B, L, C = 64, 8192, 8
M = 16384
NCORES = 8
SPC = B // NCORES  # samples per core

FBSCALE = 1.0 / 32.0
HSCALE = 1.0 / 16.0
KSCALE = 1.0 / 32.0  # HSCALE*KSCALE = (1/M) * (1/FBSCALE)

# ---------------------------------------------------------------------------
# constant tables (input-independent)
# ---------------------------------------------------------------------------


def _f16(x):
    return np.ascontiguousarray(x, dtype=np.float16)


def _rep8(x):
    return np.tile(x, (1, C))


def _build_const_tables():
    j = np.arange(128)
    D = np.exp(-2j * np.pi * np.outer(j, j) / 128.0)  # symmetric
    Dc = np.conj(D)
    Wt = np.exp(-2j * np.pi * np.outer(j, j) / M)  # fwd twiddle [n1,k2]
    W2 = np.conj(Wt)  # inv twiddle
    Dr, Di = D.real, D.imag
    Hr, Hi = (Dc * HSCALE).real, (Dc * HSCALE).imag
    Kr, Ki = (Dc * KSCALE).real[:, :64], (Dc * KSCALE).imag[:, :64]

    cols = []
    # F (fwd stage2, Karatsuba combine): F1,F2,F2n,F3,F4 [128,128]
    cols += [Dr + Di, Dr - Di, Di - Dr, -Di, Dr]
    # H (ifft stage1, plain complex): H_A=[Hr|Hi], H_B=[-Hi|Hr] [128,256]
    cols += [np.concatenate([Hr, Hi], 1)]
    cols += [np.concatenate([-Hi, Hr], 1)]
    # K (ifft stage2, Karatsuba): K1,K2,K2n,K3,K4 [128,64]
    cols += [Kr + Ki, Kr - Ki, Ki - Kr, -Ki, Kr]
    # twiddle tables replicated x8 channel pages (keeps DVE in 2x mode)
    cols += [_rep8(Wt.real), _rep8(-Wt.imag), _rep8(Wt.real + Wt.imag)]
    cols += [_rep8(W2.real), _rep8(-W2.imag), _rep8(W2.real + W2.imag)]
    ca = _f16(np.concatenate(cols, axis=1))

    cb = _f16(
        np.concatenate([Dr[:64], Di[:64], -Di[:64], Dr[:64]], axis=1)
    )  # [64, 512] = Dtab1|Dtab2
    return ca, cb


# column offsets in ca
_F = [0, 128, 256, 384, 512]  # F1,F2,F2n,F3,F4
_HA, _HB = 640, 896
_K = [1152, 1216, 1280, 1344, 1408]  # K1,K2,K2n,K3,K4
_WR, _WNI, _WS = 1472, 2496, 3520  # [128, 1024] each (replicated x8)
_W2R, _W2NI, _W2S = 4544, 5568, 6592
CA1_COLS = 4544
CA_COLS = 7616

_CONST_CACHE = {}


def _consts():
    if "ca" not in _CONST_CACHE:
        ca, cb = _build_const_tables()
        assert ca.shape[1] == CA_COLS, ca.shape
        _CONST_CACHE["ca"] = ca
        _CONST_CACHE["cb"] = cb
    return _CONST_CACHE["ca"], _CONST_CACHE["cb"]


# ---------------------------------------------------------------------------
# device module
# ---------------------------------------------------------------------------

_MODULE_CACHE = {}


def _build_module():
    import concourse.bass as bass
    from concourse import mybir

    dt = mybir.dt
    NB = 2  # per-sample buffer depth

    nc = bass.Bass("TRN2", target_bir_lowering=False, debug=False)

    xt = nc.dram_tensor("xt", [SPC, C, L], dt.float16, kind="ExternalInput").ap()
    # chirp tables, already replicated x8 channels: [SPC, 2, 64, 1024]
    ach = nc.dram_tensor("ach", [SPC, 2, 64, 1024], dt.float16, kind="ExternalInput").ap()
    # Fb planes (replicated x8 ch): [SPC, 3, 128, 1024] = (Fbr, +Fbi, -Fbi)*FBSCALE
    fbd = nc.dram_tensor("fbd", [SPC, 3, 128, 1024], dt.float16, kind="ExternalInput").ap()
    cad = nc.dram_tensor("cad", [128, CA_COLS], dt.float16, kind="ExternalInput").ap()
    cbd = nc.dram_tensor("cbd", [64, 512], dt.float16, kind="ExternalInput").ap()
    outr = nc.dram_tensor(
        "outr", [SPC, C, 2, L], dt.float16, kind="ExternalOutput"
    ).ap()

    ctx_list = []

    def sb(name, shape, dtype=None):
        t = nc.sbuf_tensor(name, shape, dtype or mybir.dt.float16)
        ap = t.__enter__()
        ctx_list.append(t)
        return ap

    def psum(name, shape):
        t = nc.psum_tensor(name, shape, mybir.dt.float32)
        ap = t.__enter__()
        ctx_list.append(t)
        return ap

    ca = sb("ca", [128, CA_COLS])
    cb = sb("cb", [64, 512])
    x_t = [sb(f"x{i}", [64, 1024]) for i in range(NB)]
    ach_t = [sb(f"ach{i}", [64, 2048]) for i in range(NB)]
    fbR = [sb(f"fbR{i}", [128, 3072]) for i in range(NB)]
    A_t = [sb(f"A{i}", [64, 2048]) for i in range(NB)]
    Yf = [sb(f"Yf{i}", [128, 2048]) for i in range(NB)]
    Pb = [sb(f"Pb{i}", [128, 1024]) for i in range(NB)]
    Qnb = [sb(f"Qnb{i}", [128, 1024]) for i in range(NB)]
    Rb = [sb(f"Rb{i}", [128, 1024]) for i in range(NB)]
    Ff = [sb(f"Ff{i}", [128, 2048]) for i in range(NB)]
    CRb = [sb(f"CRb{i}", [128, 1024]) for i in range(NB)]
    CIb = [sb(f"CIb{i}", [128, 1024]) for i in range(NB)]
    Sf = [sb(f"Sf{i}", [128, 2048]) for i in range(NB)]
    P3b = [sb(f"P3b{i}", [128, 1024]) for i in range(NB)]
    Q3nb = [sb(f"Q3nb{i}", [128, 1024]) for i in range(NB)]
    R3b = [sb(f"R3b{i}", [128, 1024]) for i in range(NB)]
    ob = [sb(f"ob{i}", [64, 2048]) for i in range(NB)]
    # single-buffered DVE scratch (same-engine producer/consumer) -- but with
    # pair interleaving two samples' DVE groups are adjacent, so double them
    S1 = [sb(f"S1_{i}", [128, 1024]) for i in range(NB)]
    S3 = [sb(f"S3_{i}", [128, 1024]) for i in range(NB)]
    M1 = [sb(f"M1_{i}", [128, 1024]) for i in range(NB)]
    M2 = [sb(f"M2_{i}", [128, 1024]) for i in range(NB)]
    M3 = [sb(f"M3_{i}", [128, 1024]) for i in range(NB)]
    M4 = [sb(f"M4_{i}", [128, 1024]) for i in range(NB)]

    # two 4-bank psum regions; samples alternate regions by parity, and each
    # region runs its own strict phase sequence s1 -> s2 -> is1 -> is2
    psR = [psum("psR0", [128, 2048]), psum("psR1", [128, 2048])]

    csem = nc.alloc_semaphore("csem")
    c2sem = nc.alloc_semaphore("c2sem")
    cbsem = nc.alloc_semaphore("cbsem")
    smp = [nc.alloc_semaphore(f"smp{i}") for i in range(SPC)]
    osem = [nc.alloc_semaphore(f"osem{i}") for i in range(SPC)]
    fsem = [nc.alloc_semaphore(f"fsem{i}") for i in range(SPC)]
    vsem = nc.alloc_semaphore("vsem")
    psem = nc.alloc_semaphore("psem")
    ssem = nc.alloc_semaphore("ssem")
    gsem = nc.alloc_semaphore("gsem")

    # ---- emission orders (pair-interleaved) and semaphore target tables ----
    pairs = [(2 * p, 2 * p + 1) for p in range(SPC // 2)]

    pe_order = []   # (phase, s), phase in 0..3
    act_order = []  # (evac, s)
    dve_order = []  # (group, s), group in 0..2 (L1, CL, L3)
    gp_order = []   # (kind, s), kind 0=a-planes, 1=fbR
    for (sa, sb_) in pairs:
        for ph in range(4):
            pe_order += [(ph, sa), (ph, sb_)]
            act_order += [(ph, sa), (ph, sb_)]
        for g in range(3):
            dve_order += [(g, sa), (g, sb_)]
        gp_order += [(0, sa), (0, sb_)]
    gp_order = [e for e in gp_order if e[1] >= 2]

    PSEM = {}
    for i, key in enumerate(pe_order):
        PSEM[key] = i + 1
    SSEM = {}
    for i, key in enumerate(act_order):
        SSEM[key] = i + 1
    GSEM = {}
    g = 0
    for kind, s in gp_order:
        g += 2
        GSEM[(kind, s)] = g
    # DVE op positions per group (emission order below):
    #  L1/L3: P(+1), Qn(+2), S(+3), R(+4);  CL: M1,M2,CR,M3,M4,CI (+1..+6)
    VSEM = {}
    VOP = {}
    v = 4  # 4 startup a-plane ops on DVE (samples 0,1)
    for grp, s in dve_order:
        nops = (4, 6, 4)[grp]
        for k in range(1, nops + 1):
            VOP[(grp, s, k)] = v + k
        v += nops
        VSEM[(grp, s)] = v

    AluOp = mybir.AluOpType

    with nc.Block() as block:

        @block.sync
        def _(sync):
            def emit_in(s):
                b = s % NB
                if s >= NB:
                    if s - NB < 2:
                        sync.wait_ge(vsem, 2 * (s - NB + 1))
                    else:
                        sync.wait_ge(gsem, GSEM[(0, s - NB)])
                    sync.wait_ge(vsem, VSEM[(1, s - NB)])
                sync.dma_start(
                    x_t[b][:].rearrange("p (c n) -> p c n", c=C),
                    xt[s].rearrange("c (p n) -> p c n", n=128),
                ).then_inc(smp[s], 16)
                sync.dma_start(
                    ach_t[b][:].rearrange("p (r n) -> p r n", r=2),
                    ach[s].rearrange("r p n -> p r n"),
                ).then_inc(smp[s], 16)
                sync.dma_start(
                    fbR[b][:].rearrange("p (f n) -> p f n", f=3),
                    fbd[s].rearrange("f p n -> p f n"),
                ).then_inc(fsem[s], 16)

            def emit_out(s):
                b = s % NB
                sync.wait_ge(ssem, SSEM[(3, s)])
                obv = ob[b][:].rearrange("p (q v) -> p q v", q=4)
                orv = outr[s].rearrange("(q j) r (p n) -> p q j r n", q=4, n=128)
                for r in range(2):
                    for jj in range(2):
                        sync.dma_start(
                            orv[:, :, jj, r, :],
                            obv[:, :, 256 * r + 128 * jj : 256 * r + 128 * jj + 128],
                        ).then_inc(osem[s], 16)

            # startup: tiny cb first, then sample-0 x/ach so Pool can start,
            # then the two halves of the big constant table between loads
            sync.dma_start(cb[:], cbd[:]).then_inc(cbsem, 16)
            sync.dma_start(
                x_t[0][:].rearrange("p (c n) -> p c n", c=C),
                xt[0].rearrange("c (p n) -> p c n", n=128),
            ).then_inc(smp[0], 16)
            sync.dma_start(
                ach_t[0][:].rearrange("p (r n) -> p r n", r=2),
                ach[0].rearrange("r p n -> p r n"),
            ).then_inc(smp[0], 16)
            sync.dma_start(ca[:, 0:CA1_COLS], cad[:, 0:CA1_COLS]).then_inc(csem, 16)
            sync.dma_start(
                fbR[0][:].rearrange("p (f n) -> p f n", f=3),
                fbd[0].rearrange("f p n -> p f n"),
            ).then_inc(fsem[0], 16)
            emit_in(1)
            sync.dma_start(ca[:, CA1_COLS:], cad[:, CA1_COLS:]).then_inc(c2sem, 16)
            emit_in(2)
            emit_in(3)
            for s in range(SPC):
                if s + 4 < SPC:
                    emit_in(s + 4)
                emit_out(s)

        @block.gpsimd
        def _(gp):
            for kind, s in gp_order:
                b = s % NB
                gp.wait_ge(smp[s], 32)
                if s >= NB:
                    gp.wait_ge(psem, PSEM[(0, s - NB)])  # A_t[b] free
                nc.gpsimd.tensor_tensor(
                    A_t[b][:, 0:1024], x_t[b][:], ach_t[b][:, 0:1024], AluOp.mult
                ).then_inc(gsem, 1)
                nc.gpsimd.tensor_tensor(
                    A_t[b][:, 1024:2048],
                    x_t[b][:],
                    ach_t[b][:, 1024:2048],
                    AluOp.mult,
                ).then_inc(gsem, 1)

        @block.vector
        def _(vector):
            def chpages(ap):
                v_ = ap.rearrange("p (c u) -> p c u", c=C)
                return v_[:, :, 0:128], v_[:, :, 128:256]

            def prpages(ap):
                v_ = ap.rearrange("p (q u) -> p q u", q=4)
                return v_[:, :, 0:256], v_[:, :, 256:512]

            def flat4(ap):
                return ap.rearrange("p (q u) -> p q u", q=4)

            def flat8(ap):
                return ap.rearrange("p (c u) -> p c u", c=C)

            for s0 in (0, 1):
                vector.wait_ge(smp[s0], 32)
                nc.vector.tensor_tensor(
                    A_t[s0][:, 0:1024], x_t[s0][:], ach_t[s0][:, 0:1024], AluOp.mult
                ).then_inc(vsem, 1)
                nc.vector.tensor_tensor(
                    A_t[s0][:, 1024:2048],
                    x_t[s0][:],
                    ach_t[s0][:, 1024:2048],
                    AluOp.mult,
                ).then_inc(vsem, 1)
            first_dve = [True]
            for grp, s in dve_order:
                if first_dve[0]:
                    vector.wait_ge(csem, 16)
                    first_dve[0] = False
                    first_l3 = [True]
                b = s % NB
                if grp == 0:
                    # L1 (fwd twiddle, Karatsuba planes) from Yf
                    vector.wait_ge(ssem, SSEM[(0, s)])
                    if s >= NB:
                        vector.wait_ge(psem, PSEM[(1, s - NB)])  # Pb/Qnb/Rb free
                    yre, yim = chpages(Yf[b][:])
                    nc.vector.tensor_tensor(
                        flat8(Pb[b][:]), yre, flat8(ca[:, _WR : _WR + 1024]), AluOp.mult
                    ).then_inc(vsem, 1)
                    nc.vector.tensor_tensor(
                        flat8(Qnb[b][:]),
                        yim,
                        flat8(ca[:, _WNI : _WNI + 1024]),
                        AluOp.mult,
                    ).then_inc(vsem, 1)
                    nc.vector.tensor_tensor(
                        flat8(S1[b][:]), yre, yim, AluOp.add
                    ).then_inc(vsem, 1)
                    vector.wait_ge(vsem, VOP[(grp, s, 3)])  # S1 drained
                    nc.vector.tensor_tensor(
                        Rb[b][:], S1[b][:], ca[:, _WS : _WS + 1024], AluOp.mult
                    ).then_inc(vsem, 1)
                elif grp == 1:
                    # C-layer (Fa o Fb, plain complex) from Ff (pair-major)
                    vector.wait_ge(ssem, SSEM[(1, s)])
                    vector.wait_ge(fsem[s], 16)
                    fre, fim = prpages(Ff[b][:])
                    nc.vector.tensor_tensor(
                        flat4(M1[b][:]), fre, flat4(fbR[b][:, 0:1024]), AluOp.mult
                    ).then_inc(vsem, 1)
                    nc.vector.tensor_tensor(
                        flat4(M2[b][:]), fim, flat4(fbR[b][:, 2048:3072]), AluOp.mult
                    ).then_inc(vsem, 1)
                    vector.wait_ge(vsem, VOP[(grp, s, 2)])  # M1, M2 drained
                    if s >= NB:
                        vector.wait_ge(psem, PSEM[(2, s - NB)])  # CRb/CIb free
                    nc.vector.tensor_tensor(
                        CRb[b][:], M1[b][:], M2[b][:], AluOp.add
                    ).then_inc(vsem, 1)
                    nc.vector.tensor_tensor(
                        flat4(M3[b][:]), fre, flat4(fbR[b][:, 1024:2048]), AluOp.mult
                    ).then_inc(vsem, 1)
                    nc.vector.tensor_tensor(
                        flat4(M4[b][:]), fim, flat4(fbR[b][:, 0:1024]), AluOp.mult
                    ).then_inc(vsem, 1)
                    vector.wait_ge(vsem, VOP[(grp, s, 5)])  # M3, M4 drained
                    nc.vector.tensor_tensor(
                        CIb[b][:], M3[b][:], M4[b][:], AluOp.add
                    ).then_inc(vsem, 1)
                else:
                    # L3 (inv twiddle, Karatsuba planes) from Sf
                    if first_l3[0]:
                        vector.wait_ge(c2sem, 16)
                        first_l3[0] = False
                    vector.wait_ge(ssem, SSEM[(2, s)])
                    if s >= NB:
                        vector.wait_ge(psem, PSEM[(3, s - NB)])  # P3b/.. free
                    sre, sim_ = chpages(Sf[b][:])
                    nc.vector.tensor_tensor(
                        flat8(P3b[b][:]),
                        sre,
                        flat8(ca[:, _W2R : _W2R + 1024]),
                        AluOp.mult,
                    ).then_inc(vsem, 1)
                    nc.vector.tensor_tensor(
                        flat8(Q3nb[b][:]),
                        sim_,
                        flat8(ca[:, _W2NI : _W2NI + 1024]),
                        AluOp.mult,
                    ).then_inc(vsem, 1)
                    nc.vector.tensor_tensor(
                        flat8(S3[b][:]), sre, sim_, AluOp.add
                    ).then_inc(vsem, 1)
                    vector.wait_ge(vsem, VOP[(grp, s, 3)])  # S3 drained
                    nc.vector.tensor_tensor(
                        R3b[b][:], S3[b][:], ca[:, _W2S : _W2S + 1024], AluOp.mult
                    ).then_inc(vsem, 1)

        @block.tensor
        def _(tensor):
            mm = nc.tensor.matmul
            first_pe = [True]

            def phase_s1(s):
                b = s % NB
                ps = psR[s % 2]
                if first_pe[0]:
                    tensor.wait_ge(cbsem, 16)  # cb loaded
                    first_pe[0] = False
                if s < 2:
                    tensor.wait_ge(vsem, 2 * (s + 1))  # startup a-planes on DVE
                else:
                    tensor.wait_ge(gsem, GSEM[(0, s)])
                if s >= NB:
                    tensor.wait_ge(ssem, SSEM[(3, s - NB)])  # region free
                for c in range(C):
                    o = ps[:, 256 * c : 256 * c + 256]
                    mm(
                        o,
                        A_t[b][:, 128 * c : 128 * c + 128],
                        cb[:, 0:256],
                        start=True,
                        stop=False,
                    )
                    i = mm(
                        o,
                        A_t[b][:, 1024 + 128 * c : 1024 + 128 * c + 128],
                        cb[:, 256:512],
                        start=False,
                        stop=True,
                    )
                    if c == C - 1:
                        i.then_inc(psem, 1)

            def phase_s2(s):
                b = s % NB
                ps = psR[s % 2]
                tensor.wait_ge(vsem, VOP[(0, s, 1)])  # Pb ready
                tensor.wait_ge(csem, 16)  # ca loaded
                tensor.wait_ge(ssem, SSEM[(0, s)])  # region free after evacY
                srcs = [
                    (Pb[b], _F[0], 0, True, False, None),
                    (Qnb[b], _F[0], 256, False, False, VOP[(0, s, 2)]),
                    (Qnb[b], _F[1], 0, False, False, None),
                    (Pb[b], _F[2], 256, False, False, None),
                    (Rb[b], _F[3], 0, False, False, VOP[(0, s, 4)]),
                    (Rb[b], _F[4], 256, False, True, None),
                ]
                for wi, (buf, fofs, oofs, st, sp, wv) in enumerate(srcs):
                    if wv is not None:
                        tensor.wait_ge(vsem, wv)
                    for q in range(4):
                        i = mm(
                            ps[:, 512 * q + oofs : 512 * q + oofs + 256],
                            ca[:, fofs : fofs + 128],
                            buf[:, 256 * q : 256 * q + 256],
                            start=st,
                            stop=sp,
                        )
                        if wi == 5 and q == 3:
                            i.then_inc(psem, 1)

            def phase_is1(s):
                b = s % NB
                ps = psR[s % 2]
                tensor.wait_ge(vsem, VOP[(1, s, 3)])  # CRb ready
                tensor.wait_ge(ssem, SSEM[(1, s)])
                for c in range(C):
                    # even channel opens its bank; odd writes the other half
                    mm(
                        ps[:, 256 * c : 256 * c + 256],
                        CRb[b][:, 128 * c : 128 * c + 128],
                        ca[:, _HA : _HA + 256],
                        start=(c % 2 == 0),
                        stop=False,
                    )
                tensor.wait_ge(vsem, VOP[(1, s, 6)])  # CIb ready
                for c in range(C):
                    i = mm(
                        ps[:, 256 * c : 256 * c + 256],
                        CIb[b][:, 128 * c : 128 * c + 128],
                        ca[:, _HB : _HB + 256],
                        start=False,
                        stop=(c % 2 == 1),
                    )
                    if c == C - 1:
                        i.then_inc(psem, 1)

            def phase_is2(s):
                b = s % NB
                ps = psR[s % 2]
                tensor.wait_ge(vsem, VOP[(2, s, 1)])  # P3b ready
                tensor.wait_ge(ssem, SSEM[(2, s)])
                srcs = [
                    (P3b[b], _K[0], 0, True, False, None),
                    (Q3nb[b], _K[0], 256, False, False, VOP[(2, s, 2)]),
                    (Q3nb[b], _K[1], 0, False, False, None),
                    (P3b[b], _K[2], 256, False, False, None),
                    (R3b[b], _K[3], 0, False, False, VOP[(2, s, 4)]),
                    (R3b[b], _K[4], 256, False, True, None),
                ]
                for wi, (buf, kofs, oofs, st, sp, wv) in enumerate(srcs):
                    if wv is not None:
                        tensor.wait_ge(vsem, wv)
                    for q in range(4):
                        i = mm(
                            ps[0:64, 512 * q + oofs : 512 * q + oofs + 256],
                            ca[:, kofs : kofs + 64],
                            buf[:, 256 * q : 256 * q + 256],
                            start=st,
                            stop=sp,
                        )
                        if wi == 5 and q == 3:
                            i.then_inc(psem, 1)

            phase_fns = [phase_s1, phase_s2, phase_is1, phase_is2]
            for ph, s in pe_order:
                phase_fns[ph](s)

        @block.scalar
        def _(scalar):
            for ph, s in act_order:
                b = s % NB
                ps = psR[s % 2]
                scalar.wait_ge(psem, PSEM[(ph, s)])
                if ph == 0:
                    nc.scalar.copy(Yf[b][:], ps[:, 0:2048]).then_inc(ssem, 1)
                elif ph == 1:
                    nc.scalar.copy(Ff[b][:], ps[:, 0:2048]).then_inc(ssem, 1)
                elif ph == 2:
                    nc.scalar.copy(Sf[b][:], ps[:, 0:2048]).then_inc(ssem, 1)
                else:
                    if s >= NB:
                        scalar.wait_ge(osem[s - NB], 64)
                    nc.scalar.copy(ob[b][:], ps[0:64, 0:2048]).then_inc(ssem, 1)

    for t in reversed(ctx_list):
        t.__exit__(None, None, None)

    return nc


def _get_module():
    if "nc" not in _MODULE_CACHE:
        _MODULE_CACHE["nc"] = _build_module()
    return _MODULE_CACHE["nc"]


# ---------------------------------------------------------------------------
# host side
# ---------------------------------------------------------------------------


def _host_tables(rpm):
    """Per-sample chirp tables (replicated x8 channels) + Fb planes."""
    pad = np.floor((RES * 60.0 / rpm.astype(np.float64) - TS) * SF).astype(np.int64)
    n_arr = L + pad
    t = np.arange(L, dtype=np.int64)
    m = np.arange(M, dtype=np.int64)
    mm = np.minimum(m, M - m)

    ach = np.empty((B, 2, 64, 1024), np.float16)
    fbp = np.empty((B, 3, 128, 1024), np.float16)
    for b in range(B):
        n = int(n_arr[b])
        two_n = 2 * n
        ph = np.pi * ((t * t) % two_n) / n
        cosg = np.cos(ph).astype(np.float16).reshape(64, 128)
        nsing = (-np.sin(ph)).astype(np.float16).reshape(64, 128)
        ach[b, 0] = np.tile(cosg, (1, C))
        ach[b, 1] = np.tile(nsing, (1, C))
        phb = np.pi * ((mm * mm) % two_n) / n
        Fb = np.fft.fft(np.exp(1j * phb)).reshape(128, 128) * FBSCALE
        fbp[b, 0] = np.tile(Fb.real.astype(np.float16), (1, C))
        fbp[b, 1] = np.tile(Fb.imag.astype(np.float16), (1, C))
        fbp[b, 2] = np.tile((-Fb.imag).astype(np.float16), (1, C))
    return ach, fbp


LAST_EXEC_WALL_NS = [None]


def kernel(inputs, rpm):
    inputs = np.ascontiguousarray(inputs, dtype=np.float32)  # [B, L, C]
    rpm = np.ascontiguousarray(rpm, dtype=np.float32)

    ca, cb = _consts()
    ach, fbp = _host_tables(rpm)
    xt = np.ascontiguousarray(inputs.transpose(0, 2, 1)).astype(np.float16)

    nc = _get_module()
    in_maps = []
    for g in range(NCORES):
        s0 = g * SPC
        in_maps.append(
            {
                "xt": xt[s0 : s0 + SPC],
                "ach": ach[s0 : s0 + SPC],
                "fbd": fbp[s0 : s0 + SPC],
                "cad": ca,
                "cbd": cb,
            }
        )

    import time

    from concourse.bass_utils import run_bass_kernel_spmd

    t0 = time.perf_counter_ns()
    res = run_bass_kernel_spmd(nc, in_maps, list(range(NCORES)))
    LAST_EXEC_WALL_NS[0] = time.perf_counter_ns() - t0

    out = np.empty((B, L, C), np.float32)
    for g in range(NCORES):
        planes = np.asarray(res.results[g]["outr"], np.float32)  # [SPC, C, 2, L]
        mag = np.hypot(planes[:, :, 0, :], planes[:, :, 1, :])  # [SPC, C, L]
        out[g * SPC : (g + 1) * SPC] = mag.transpose(0, 2, 1)
    return out
